# revision 41
# baseline (speedup 1.0000x reference)
"""Local-window multi-head attention (window=33) for Trainium2, 8-core SPMD.

Sharding: data-parallel over batch (B=8 -> 1 batch per core). Weights
replicated. Per core: QKV projections, banded local attention via
transposed-score blocks of 96 queries x 128 keys, output projection,
fused in one Bass/Tile kernel.

Layout notes:
  - Q/K/V projection path in fp16 (x^T staged fp16, weights fp16);
    MM1 contracts K=128 with the co-projected head's partition half
    zeroed in qZ (base-64 matmul operands fault at runtime).
  - attention path (exp probs, v tiles, attn, Wo) in bf16 -- exp output
    needs bf16 range (scores reach ~40, e^40 overflows fp16).
  - scores computed transposed: S^T[kpos, q] = k_h^T.T @ q_h^T, so the
    key-side mask/range penalty is a per-partition bias of the exp on
    ScalarE, and MM2 (P^T as lhsT) gives attn natural [q, d] with the
    softmax denominator from a ones-column appended to v.
  - v projected per 96-query block directly into [kpos 128, head, 66]
    tiles (cols 64:66 = ones via Pool memset), so MM2 needs no K-splits.
  - band mask applied on GpSimd; per-block transposed attn collected
    into a per-chunk supertile so the output projection runs at M=128
    (CPB=8: 768 queries = 6 supertiles; last chunk 256 = 2).
  - output bias (Wo@bv + bo, constant row) added on host; out stored
    fp16 and upcast on host.
"""
import contextlib
import os
import sys
sys.path.insert(0, "/opt/trn_rl_repo")
import numpy as np

B, S, D, H, HD = 8, 4096, 512, 8, 64
WIN, HALF = 33, 16
QB = 96
NB = (S + QB - 1) // QB          # 43 blocks (42 full + 64)
CPB = 8                          # blocks per chunk (768 queries = 6 x 128-row
                                 # output supertiles; last chunk 256 = 2 x 128)
NEG = -1e9

_CHUNKS = [list(range(c * CPB, min(NB, (c + 1) * CPB))) for c in range((NB + CPB - 1) // CPB)]

_NCS = {}
REPS = int(os.environ.get("BASS_KERNEL_REPS", "1"))


def _qw(j):
    return min(QB, S - QB * j)


def _build(reps=None):
    reps = REPS if reps is None else reps
    import concourse.bacc as bacc
    import concourse.mybir as mybir
    from concourse.tile import TileContext

    F32 = mybir.dt.float32
    F16 = mybir.dt.float16
    BF16 = mybir.dt.bfloat16
    EXP = mybir.ActivationFunctionType.Exp
    MULT = mybir.AluOpType.mult

    nc = bacc.Bacc(None, target_bir_lowering=False)

    xqT = nc.dram_tensor("xqT", [D, S], F16, kind="ExternalInput")
    xkT = nc.dram_tensor("xkT", [D, S], F16, kind="ExternalInput")
    xvT = nc.dram_tensor("xvT", [D, S], F16, kind="ExternalInput")
    wqkvT = nc.dram_tensor("wqkvT", [3 * D, D], F16, kind="ExternalInput")
    woT = nc.dram_tensor("woT", [D, D], BF16, kind="ExternalInput")
    cf32_d = nc.dram_tensor("cf32", [128, 8 + NB], F32, kind="ExternalInput")
    cbf_d = nc.dram_tensor("cbf", [128, QB + 128], BF16, kind="ExternalInput")
    zpad_d = nc.dram_tensor("zpad", [128, 4, 64], F16, kind="ExternalInput")
    out_d = nc.dram_tensor("out", [S, D], F16, kind="ExternalOutput")

    def r4(t):  # [512, N] dram -> [128, 4, N] view
        return t[:, :].rearrange("(c p) n -> p c n", p=128)

    with TileContext(nc) as tc:
        with tc.tile_pool(name="const", bufs=1) as cp, \
             tc.tile_pool(name="stage", bufs=2) as stp, \
             tc.tile_pool(name="qk", bufs=2) as qkp, \
             tc.tile_pool(name="vtiles", bufs=2 * CPB + 2) as vp, \
             tc.tile_pool(name="pt", bufs=6) as ptp, \
             tc.tile_pool(name="small", bufs=3) as smp, \
             tc.tile_pool(name="outp", bufs=3) as outp, \
             tc.tile_pool(name="proj_ps", bufs=2, space="PSUM") as proj_ps, \
             tc.tile_pool(name="st_ps", bufs=2, space="PSUM") as st_ps, \
             tc.tile_pool(name="mm2_ps", bufs=2, space="PSUM") as mm2_ps, \
             tc.tile_pool(name="tr_ps", bufs=1, space="PSUM") as tr_ps, \
             tc.tile_pool(name="op_ps", bufs=1, space="PSUM") as op_ps:

            # ---- constants (packed into few DMAs to shorten the prologue) ----
            w3_sb = cp.tile([128, 3, 4, D], F16, name="w3_sb")
            wo_sb = cp.tile([128, 4, D], BF16, name="wo_sb")
            nc.sync.dma_start(
                w3_sb[:], wqkvT[:, :].rearrange("(w c p) n -> p w c n", p=128, c=4))
            nc.sync.dma_start(wo_sb[:], r4(woT))
            wq_sb, wk_sb, wv_sb = w3_sb[:, 0], w3_sb[:, 1], w3_sb[:, 2]
            cf32 = cp.tile([128, 8 + NB], F32, name="cf32_sb")
            cbf = cp.tile([128, QB + 128], BF16, name="cbf_sb")
            nc.sync.dma_start(cf32[:], cf32_d[:, :])
            nc.sync.dma_start(cbf[:], cbf_d[:, :])
            bqc, bkc, pen = cf32[:, 0:4], cf32[:, 4:8], cf32[:, 8:8 + NB]
            band, iden = cbf[:, 0:QB], cbf[:, QB:QB + 128]

            # persistent double-buffered qZ: per-head q^T with the other
            # co-projected head's partition half zeroed (lets MM1 run as a
            # full-K=128 matmul at partition base 0 -- base-64 matmul
            # operands fault at runtime)
            qZ_bufs = []
            for bi in range(2):
                qz = cp.tile([128, H, CPB * QB], F16, name=f"qZ{bi}")
                nc.gpsimd.memset(qz[0:64, 1:H:2, :], 0.0)
                nc.gpsimd.memset(qz[64:128, 0:H:2, :], 0.0)
                qZ_bufs.append(qz)

            NC = len(_CHUNKS)
            stg_map, kT_map, vts_map = {}, {}, {}

            def chunk_geom(ci):
                blocks = _CHUNKS[ci]
                j0, j1 = blocks[0], blocks[-1]
                q_lo = QB * j0
                qwid = min(S, QB * (j1 + 1)) - q_lo          # 768 / 256
                win_lo = QB * j0 - HALF                      # may be < 0
                win_hi = QB * j1 + 112                       # may be > S
                return blocks, q_lo, qwid, win_lo, win_hi

            def stage_chunk(ci):
                blocks, q_lo, qwid, win_lo, win_hi = chunk_geom(ci)
                kwid = win_hi - win_lo
                dlo, dhi = max(0, win_lo), min(S, win_hi)
                xq_st = stp.tile([128, 4, CPB * QB], F16, tag="xq_st", name="xq_st")
                xk_st = stp.tile([128, 4, CPB * QB + 32], F16, tag="xk_st", name="xk_st")
                xv_st = stp.tile([128, 4, CPB * QB + 32], F16, tag="xv_st", name="xv_st")
                nc.sync.dma_start(xq_st[:, :, :qwid], r4(xqT)[:, :, q_lo:q_lo + qwid])
                nc.sync.dma_start(xk_st[:, :, dlo - win_lo:dhi - win_lo],
                                  r4(xkT)[:, :, dlo:dhi])
                nc.sync.dma_start(xv_st[:, :, dlo - win_lo:dhi - win_lo],
                                  r4(xvT)[:, :, dlo:dhi])
                if dlo > win_lo:
                    w = dlo - win_lo
                    nc.sync.dma_start(xk_st[:, :, 0:w], zpad_d[:, :, 0:w])
                    nc.sync.dma_start(xv_st[:, :, 0:w], zpad_d[:, :, 0:w])
                if dhi < win_hi:
                    w = win_hi - dhi
                    nc.sync.dma_start(xk_st[:, :, dhi - win_lo:kwid],
                                      zpad_d[:, :, 0:w])
                    nc.sync.dma_start(xv_st[:, :, dhi - win_lo:kwid],
                                      zpad_d[:, :, 0:w])
                stg_map[ci] = (xq_st, xk_st, xv_st)

            def proj_groups(ci):
                """List of closures, each emitting one projection PSUM-tile
                group. Interleaved into the previous chunk's attention loop to
                fill PE dependency-stall gaps with independent matmuls."""
                blocks, q_lo, qwid, win_lo, win_hi = chunk_geom(ci)
                kwid = win_hi - win_lo
                xq_st, xk_st, xv_st = stg_map[ci]
                qZ = qZ_bufs[ci % 2]
                kT = qkp.tile([128, 4, CPB * QB + 32], F16, tag="kT", name="kT")
                kT_map[ci] = kT
                vts_map[ci] = {}
                groups = []
                for is_q, src, w_sb, b_sb, wid in (
                        (True, xq_st, wq_sb, bqc, qwid),
                        (False, xk_st, wk_sb, bkc, kwid)):
                    ntile = -(-wid // 512)
                    nw = wid // ntile
                    for dc in range(4):
                        for t in range(ntile):
                            def qk_group(is_q=is_q, src=src, w_sb=w_sb, b_sb=b_sb,
                                         dc=dc, to=nw * t, nw=nw, qZ=qZ, kT=kT):
                                ps = proj_ps.tile([128, 512], F32, tag="proj",
                                                  name="pps")
                                for k in range(4):
                                    nc.tensor.matmul(
                                        ps[:, :nw],
                                        w_sb[:, k, 128 * dc:128 * dc + 128],
                                        src[:, k, to:to + nw],
                                        start=(k == 0), stop=(k == 3))
                                sl = slice(to, to + nw)
                                if is_q:
                                    # alternate engines per tile; one reader
                                    # engine per PSUM tile
                                    if (dc + to) % 2 == 0:
                                        nc.vector.tensor_scalar_add(
                                            qZ[0:64, 2 * dc, sl], ps[0:64, :nw],
                                            b_sb[0:64, dc:dc + 1])
                                        nc.vector.tensor_scalar_add(
                                            qZ[64:128, 2 * dc + 1, sl],
                                            ps[64:128, :nw],
                                            b_sb[64:128, dc:dc + 1])
                                    else:
                                        nc.scalar.add(
                                            qZ[0:64, 2 * dc, sl], ps[0:64, :nw],
                                            b_sb[0:64, dc:dc + 1])
                                        nc.scalar.add(
                                            qZ[64:128, 2 * dc + 1, sl],
                                            ps[64:128, :nw],
                                            b_sb[64:128, dc:dc + 1])
                                else:
                                    nc.scalar.add(kT[:, dc, sl], ps[:, :nw],
                                                  b_sb[:, dc:dc + 1])
                            groups.append(qk_group)
                for j in blocks:
                    def v_group(j=j, vloc=(QB * j - HALF) - win_lo, ci=ci,
                                xv_st=xv_st):
                        ps = proj_ps.tile([128, 512], F32, tag="proj", name="vps")
                        for k in range(4):
                            nc.tensor.matmul(ps[:],
                                             xv_st[:, k, vloc:vloc + 128],
                                             wv_sb[:, k, :],
                                             start=(k == 0), stop=(k == 3))
                        vt = vp.tile([128, H, 66], BF16, tag="vt", name="vt")
                        nc.vector.tensor_copy(
                            vt[:, :, 0:64],
                            ps[:].rearrange("p (h d) -> p h d", d=64))
                        nc.gpsimd.memset(vt[:, :, 64:66], 1.0)
                        vts_map[ci][j] = vt
                    groups.append(v_group)
                return groups

            loop_cm = (tc.For_i(0, reps, 1) if reps > 1
                       else contextlib.nullcontext())
            with loop_cm:
                stage_chunk(0)
                for g in proj_groups(0):
                    g()
                for ci in range(NC):
                    blocks, q_lo, qwid, win_lo, win_hi = chunk_geom(ci)
                    qZ = qZ_bufs[ci % 2]
                    kT = kT_map[ci]
                    vts = vts_map[ci]
                    nextg = None
                    if ci + 1 < NC:
                        stage_chunk(ci + 1)
                        nextg = proj_groups(ci + 1)
                    nemit = 0
                    spliced = 0
                    nsplice = len(blocks)

                    def splice():
                        # emit next-chunk proj groups at fine grain so PE has
                        # independent matmuls at every chain re-entry point
                        nonlocal nemit, spliced
                        spliced += 1
                        if nextg is None:
                            return
                        hi_g = (len(nextg) * spliced) // nsplice
                        for g_fn in nextg[nemit:hi_g]:
                            g_fn()
                        nemit = hi_g

                    # atT supertile: per-block transposed attn written at 96j
                    # column offsets; o-proj consumes 128-col slices (M=128).
                    atT = smp.tile([128, 4, qwid], BF16, tag="atT", name="atT")
                    nst = qwid // 128
                    opro_after = {}
                    acc = 0
                    t = 0
                    for jj, j in enumerate(blocks):
                        acc += _qw(j)
                        while t < nst and acc >= 128 * (t + 1):
                            opro_after.setdefault(jj, []).append(t)
                            t += 1
                    for jj, j in enumerate(blocks):
                        qw = _qw(j)
                        qloc = QB * j - q_lo
                        kloc = (QB * j - HALF) - win_lo
                        pT = ptp.tile([128, H, QB], BF16, tag="pT", name="pT")
                        gps = []
                        for g in range(2):
                            st = st_ps.tile([128, 4, QB], F32, tag="st", name="st")
                            for hi in range(4):
                                h = 4 * g + hi
                                nc.tensor.matmul(
                                    st[:, hi, :qw],
                                    kT[:, h // 2, kloc:kloc + 128],
                                    qZ[:, h, qloc:qloc + qw],
                                    start=True, stop=True)
                            nc.scalar.activation(pT[:, 4 * g:4 * g + 4, :qw],
                                                 st[:, :, :qw], EXP,
                                                 bias=pen[:, j:j + 1], scale=1.0)
                            nc.gpsimd.tensor_tensor(
                                out=pT[:, 4 * g:4 * g + 4, :qw],
                                in0=pT[:, 4 * g:4 * g + 4, :qw],
                                in1=band[:, 0:qw].unsqueeze(1).to_broadcast(
                                    (128, 4, qw)),
                                op=MULT)
                        att = smp.tile([QB, D], BF16, tag="att", name="att")
                        attv = att.rearrange("q (h d) -> q h d", d=64)
                        rc = smp.tile([QB, H], F32, tag="rc", name="rc")
                        for g in range(2):
                            m2 = mm2_ps.tile([QB, 4, 66], F32, tag="m2", name="m2")
                            for hi in range(4):
                                h = 4 * g + hi
                                nc.tensor.matmul(m2[:qw, hi, :], pT[:, h, :qw],
                                                 vts[j][:, h, :],
                                                 start=True, stop=True)
                            gps.append(m2)
                        for g in range(2):
                            nc.vector.reciprocal(rc[:qw, 4 * g:4 * g + 4],
                                                 gps[g][:qw, :, 64])
                        for g in range(2):
                            nc.vector.tensor_tensor(
                                out=attv[:qw, 4 * g:4 * g + 4, :],
                                in0=gps[g][:qw, :, 0:64],
                                in1=rc[:qw, 4 * g:4 * g + 4].unsqueeze(2).to_broadcast(
                                    (qw, 4, 64)),
                                op=MULT)
                        # transpose attn -> [dcat, q], into the chunk supertile
                        tr = tr_ps.tile([128, 4, QB], BF16, tag="tr", name="tr")
                        for i in range(4):
                            nc.tensor.transpose(tr[:, i, :qw],
                                                att[:qw, 128 * i:128 * i + 128],
                                                iden[:qw, :qw])
                        nc.vector.tensor_copy(atT[:, :, qloc:qloc + qw],
                                              tr[:, :, :qw])
                        # output projection supertiles ready after this block
                        for st_i in opro_after.get(jj, ()):
                            sl = slice(128 * st_i, 128 * st_i + 128)
                            op = op_ps.tile([128, D], F32, tag="op", name="op")
                            for i in range(4):
                                nc.tensor.matmul(op[:, :], atT[:, i, sl],
                                                 wo_sb[:, i, :],
                                                 start=(i == 0), stop=(i == 3))
                            osb = outp.tile([128, D], F16, tag="osb", name="osb")
                            nc.scalar.copy(osb[:, :], op[:, :])
                            nc.sync.dma_start(
                                out_d[q_lo + 128 * st_i:q_lo + 128 * st_i + 128, :],
                                osb[:, :])
                        splice()

    nc.finalize()
    return nc


def _host_consts():
    rr = np.arange(128)[:, None]
    qq = np.arange(QB)[None, :]
    band = (((rr - qq) >= 0) & ((rr - qq) <= 32)).astype(np.float32)
    ident = np.eye(128, dtype=np.float32)
    return band, ident


def _get_nc(reps=None):
    reps = REPS if reps is None else reps
    key = reps
    if key not in _NCS:
        _NCS[key] = _build(reps)
    return _NCS[key]


def _prep_inmaps(query, key, value, mask, Wq, bq, Wk, bk, Wv, bv, Wo, bo):
    query = np.asarray(query, np.float32)
    key = np.asarray(key, np.float32)
    value = np.asarray(value, np.float32)
    mask = np.asarray(mask)
    Wq, bq = np.asarray(Wq, np.float32), np.asarray(bq, np.float32)
    Wk, bk = np.asarray(Wk, np.float32), np.asarray(bk, np.float32)
    Wv, bv = np.asarray(Wv, np.float32), np.asarray(bv, np.float32)
    Wo, bo = np.asarray(Wo, np.float32), np.asarray(bo, np.float32)

    band, ident = _host_consts()
    jj = np.arange(NB)[None, :]
    rr = np.arange(128)[:, None]
    pos = QB * jj - HALF + rr                      # [128, NB]
    valid = (pos >= 0) & (pos < S)
    posc = np.clip(pos, 0, S - 1)

    import ml_dtypes
    BF = ml_dtypes.bfloat16
    bqc = np.ascontiguousarray(bq.reshape(4, 128).T)
    bkc = np.ascontiguousarray(bk.reshape(4, 128).T)
    common = {
        "wqkvT": np.concatenate(
            [Wq.T, Wk.T, Wv.T], axis=0).astype(np.float16),
        "woT": np.ascontiguousarray(Wo.T).astype(BF),
        "cbf": np.concatenate([band, ident], axis=1).astype(BF),
        "zpad": np.zeros((128, 4, 64), np.float16),
    }
    from concurrent.futures import ThreadPoolExecutor

    def _one(b):
        pen = np.where(valid & ~mask[b][posc], 0.0, NEG).astype(np.float32)
        return dict(
            common,
            xqT=np.ascontiguousarray(query[b].T).astype(np.float16),
            xkT=np.ascontiguousarray(key[b].T).astype(np.float16),
            xvT=np.ascontiguousarray(value[b].T).astype(np.float16),
            cf32=np.concatenate([bqc, bkc, pen], axis=1).astype(np.float32),
        )

    with ThreadPoolExecutor(max_workers=8) as ex:
        in_maps = list(ex.map(_one, range(B)))
    return in_maps


def kernel(**inputs):
    from concourse.bass_utils import run_bass_kernel_spmd
    in_maps = _prep_inmaps(**inputs)
    res = run_bass_kernel_spmd(_get_nc(), in_maps, core_ids=list(range(8)))
    Wo = np.asarray(inputs["Wo"], np.float32)
    bv = np.asarray(inputs["bv"], np.float32)
    bo = np.asarray(inputs["bo"], np.float32)
    boeff = (Wo @ bv + bo).astype(np.float32)[None, :]
    out = np.stack([res.results[c]["out"].astype(np.float32) + boeff
                    for c in range(B)], axis=0)
    return out


# revision 44
# speedup vs baseline: 1.7169x; 1.7169x over previous
"""Local-window multi-head attention (window=33) for Trainium2, 8-core SPMD.

Sharding: data-parallel over batch (B=8 -> 1 batch per core). Weights
replicated. Per core: QKV projections, banded local attention via
transposed-score blocks of 96 queries x 128 keys, output projection,
fused in one Bass/Tile kernel.

Layout notes:
  - Q/K/V projection path in fp16 (x^T staged fp16, weights fp16);
    MM1 contracts K=128 with the co-projected head's partition half
    zeroed in qZ (base-64 matmul operands fault at runtime).
  - attention path (exp probs, v tiles, attn, Wo) in bf16 -- exp output
    needs bf16 range (scores reach ~40, e^40 overflows fp16).
  - scores computed transposed: S^T[kpos, q] = k_h^T.T @ q_h^T, so the
    key-side mask/range penalty is a per-partition bias of the exp on
    ScalarE, and MM2 (P^T as lhsT) gives attn natural [q, d] with the
    softmax denominator from a ones-column appended to v.
  - v projected per 96-query block directly into [kpos 128, head, 66]
    tiles (cols 64:66 = ones via Pool memset), so MM2 needs no K-splits.
  - band mask applied on GpSimd; per-block transposed attn collected
    into a per-chunk supertile so the output projection runs at M=128
    (CPB=8: 768 queries = 6 supertiles; last chunk 256 = 2).
  - output bias (Wo@bv + bo, constant row) added on host; out stored
    fp16 and upcast on host.
"""
import contextlib
import os
import sys
sys.path.insert(0, "/opt/trn_rl_repo")
import numpy as np

B, S, D, H, HD = 8, 4096, 512, 8, 64
WIN, HALF = 33, 16
QB = 96
NB = (S + QB - 1) // QB          # 43 blocks (42 full + 64)
CPB = 8                          # blocks per chunk (768 queries = 6 x 128-row
                                 # output supertiles; last chunk 256 = 2 x 128)
NEG = -1e9

_CHUNKS = [list(range(c * CPB, min(NB, (c + 1) * CPB))) for c in range((NB + CPB - 1) // CPB)]

_NCS = {}
REPS = int(os.environ.get("BASS_KERNEL_REPS", "1"))


def _qw(j):
    return min(QB, S - QB * j)


def _build(reps=None):
    reps = REPS if reps is None else reps
    import concourse.bacc as bacc
    import concourse.mybir as mybir
    from concourse.tile import TileContext

    F32 = mybir.dt.float32
    F16 = mybir.dt.float16
    BF16 = mybir.dt.bfloat16
    EXP = mybir.ActivationFunctionType.Exp
    MULT = mybir.AluOpType.mult

    nc = bacc.Bacc(None, target_bir_lowering=False)

    xqT = nc.dram_tensor("xqT", [D, S], F16, kind="ExternalInput")
    xkT = nc.dram_tensor("xkT", [D, S], F16, kind="ExternalInput")
    xvT = nc.dram_tensor("xvT", [D, S], F16, kind="ExternalInput")
    wqkvT = nc.dram_tensor("wqkvT", [3 * D, D], F16, kind="ExternalInput")
    woT = nc.dram_tensor("woT", [D, D], BF16, kind="ExternalInput")
    cf32_d = nc.dram_tensor("cf32", [128, 8 + NB], F32, kind="ExternalInput")
    cbf_d = nc.dram_tensor("cbf", [128, QB + 128], BF16, kind="ExternalInput")
    zpad_d = nc.dram_tensor("zpad", [128, 4, 64], F16, kind="ExternalInput")
    out_d = nc.dram_tensor("out", [S, D], F16, kind="ExternalOutput")

    def r4(t):  # [512, N] dram -> [128, 4, N] view
        return t[:, :].rearrange("(c p) n -> p c n", p=128)

    with TileContext(nc) as tc:
        with tc.tile_pool(name="const", bufs=1) as cp, \
             tc.tile_pool(name="stage", bufs=2) as stp, \
             tc.tile_pool(name="qk", bufs=2) as qkp, \
             tc.tile_pool(name="vtiles", bufs=2 * CPB + 2) as vp, \
             tc.tile_pool(name="pt", bufs=6) as ptp, \
             tc.tile_pool(name="small", bufs=3) as smp, \
             tc.tile_pool(name="outp", bufs=3) as outp, \
             tc.tile_pool(name="proj_ps", bufs=2, space="PSUM") as proj_ps, \
             tc.tile_pool(name="st_ps", bufs=2, space="PSUM") as st_ps, \
             tc.tile_pool(name="mm2_ps", bufs=2, space="PSUM") as mm2_ps, \
             tc.tile_pool(name="tr_ps", bufs=1, space="PSUM") as tr_ps, \
             tc.tile_pool(name="op_ps", bufs=1, space="PSUM") as op_ps:

            # ---- constants (packed into few DMAs to shorten the prologue) ----
            w3_sb = cp.tile([128, 3, 4, D], F16, name="w3_sb")
            wo_sb = cp.tile([128, 4, D], BF16, name="wo_sb")
            nc.sync.dma_start(
                w3_sb[:], wqkvT[:, :].rearrange("(w c p) n -> p w c n", p=128, c=4))
            nc.sync.dma_start(wo_sb[:], r4(woT))
            wq_sb, wk_sb, wv_sb = w3_sb[:, 0], w3_sb[:, 1], w3_sb[:, 2]
            cf32 = cp.tile([128, 8 + NB], F32, name="cf32_sb")
            cbf = cp.tile([128, QB + 128], BF16, name="cbf_sb")
            nc.sync.dma_start(cf32[:], cf32_d[:, :])
            nc.sync.dma_start(cbf[:], cbf_d[:, :])
            bqc, bkc, pen = cf32[:, 0:4], cf32[:, 4:8], cf32[:, 8:8 + NB]
            band, iden = cbf[:, 0:QB], cbf[:, QB:QB + 128]

            # persistent double-buffered qZ: per-head q^T with the other
            # co-projected head's partition half zeroed (lets MM1 run as a
            # full-K=128 matmul at partition base 0 -- base-64 matmul
            # operands fault at runtime)
            qZ_bufs = []
            for bi in range(2):
                qz = cp.tile([128, H, CPB * QB], F16, name=f"qZ{bi}")
                nc.gpsimd.memset(qz[0:64, 1:H:2, :], 0.0)
                nc.gpsimd.memset(qz[64:128, 0:H:2, :], 0.0)
                qZ_bufs.append(qz)

            NC = len(_CHUNKS)
            stg_map, kT_map, vts_map = {}, {}, {}

            def chunk_geom(ci):
                blocks = _CHUNKS[ci]
                j0, j1 = blocks[0], blocks[-1]
                q_lo = QB * j0
                qwid = min(S, QB * (j1 + 1)) - q_lo          # 768 / 256
                win_lo = QB * j0 - HALF                      # may be < 0
                win_hi = QB * j1 + 112                       # may be > S
                return blocks, q_lo, qwid, win_lo, win_hi

            def stage_chunk(ci):
                blocks, q_lo, qwid, win_lo, win_hi = chunk_geom(ci)
                kwid = win_hi - win_lo
                dlo, dhi = max(0, win_lo), min(S, win_hi)
                xq_st = stp.tile([128, 4, CPB * QB], F16, tag="xq_st", name="xq_st")
                xk_st = stp.tile([128, 4, CPB * QB + 32], F16, tag="xk_st", name="xk_st")
                xv_st = stp.tile([128, 4, CPB * QB + 32], F16, tag="xv_st", name="xv_st")
                nc.sync.dma_start(xq_st[:, :, :qwid], r4(xqT)[:, :, q_lo:q_lo + qwid])
                nc.sync.dma_start(xk_st[:, :, dlo - win_lo:dhi - win_lo],
                                  r4(xkT)[:, :, dlo:dhi])
                nc.sync.dma_start(xv_st[:, :, dlo - win_lo:dhi - win_lo],
                                  r4(xvT)[:, :, dlo:dhi])
                if dlo > win_lo:
                    w = dlo - win_lo
                    nc.sync.dma_start(xk_st[:, :, 0:w], zpad_d[:, :, 0:w])
                    nc.sync.dma_start(xv_st[:, :, 0:w], zpad_d[:, :, 0:w])
                if dhi < win_hi:
                    w = win_hi - dhi
                    nc.sync.dma_start(xk_st[:, :, dhi - win_lo:kwid],
                                      zpad_d[:, :, 0:w])
                    nc.sync.dma_start(xv_st[:, :, dhi - win_lo:kwid],
                                      zpad_d[:, :, 0:w])
                stg_map[ci] = (xq_st, xk_st, xv_st)

            def proj_groups(ci):
                """List of closures, each emitting one projection PSUM-tile
                group. Interleaved into the previous chunk's attention loop to
                fill PE dependency-stall gaps with independent matmuls."""
                blocks, q_lo, qwid, win_lo, win_hi = chunk_geom(ci)
                kwid = win_hi - win_lo
                xq_st, xk_st, xv_st = stg_map[ci]
                qZ = qZ_bufs[ci % 2]
                kT = qkp.tile([128, 4, CPB * QB + 32], F16, tag="kT", name="kT")
                kT_map[ci] = kT
                vts_map[ci] = {}
                groups = []
                for is_q, src, w_sb, b_sb, wid in (
                        (True, xq_st, wq_sb, bqc, qwid),
                        (False, xk_st, wk_sb, bkc, kwid)):
                    ntile = -(-wid // 512)
                    nw = wid // ntile
                    for dc in range(4):
                        for t in range(ntile):
                            def qk_group(is_q=is_q, src=src, w_sb=w_sb, b_sb=b_sb,
                                         dc=dc, to=nw * t, nw=nw, qZ=qZ, kT=kT):
                                ps = proj_ps.tile([128, 512], F32, tag="proj",
                                                  name="pps")
                                for k in range(4):
                                    nc.tensor.matmul(
                                        ps[:, :nw],
                                        w_sb[:, k, 128 * dc:128 * dc + 128],
                                        src[:, k, to:to + nw],
                                        start=(k == 0), stop=(k == 3))
                                sl = slice(to, to + nw)
                                if is_q:
                                    # alternate engines per tile; one reader
                                    # engine per PSUM tile
                                    if (dc + to) % 2 == 0:
                                        nc.vector.tensor_scalar_add(
                                            qZ[0:64, 2 * dc, sl], ps[0:64, :nw],
                                            b_sb[0:64, dc:dc + 1])
                                        nc.vector.tensor_scalar_add(
                                            qZ[64:128, 2 * dc + 1, sl],
                                            ps[64:128, :nw],
                                            b_sb[64:128, dc:dc + 1])
                                    else:
                                        nc.scalar.add(
                                            qZ[0:64, 2 * dc, sl], ps[0:64, :nw],
                                            b_sb[0:64, dc:dc + 1])
                                        nc.scalar.add(
                                            qZ[64:128, 2 * dc + 1, sl],
                                            ps[64:128, :nw],
                                            b_sb[64:128, dc:dc + 1])
                                else:
                                    nc.scalar.add(kT[:, dc, sl], ps[:, :nw],
                                                  b_sb[:, dc:dc + 1])
                            groups.append(qk_group)
                for j in blocks:
                    def v_group(j=j, vloc=(QB * j - HALF) - win_lo, ci=ci,
                                xv_st=xv_st):
                        ps = proj_ps.tile([128, 512], F32, tag="proj", name="vps")
                        for k in range(4):
                            nc.tensor.matmul(ps[:],
                                             xv_st[:, k, vloc:vloc + 128],
                                             wv_sb[:, k, :],
                                             start=(k == 0), stop=(k == 3))
                        vt = vp.tile([128, H, 66], BF16, tag="vt", name="vt")
                        nc.vector.tensor_copy(
                            vt[:, :, 0:64],
                            ps[:].rearrange("p (h d) -> p h d", d=64))
                        nc.gpsimd.memset(vt[:, :, 64:66], 1.0)
                        vts_map[ci][j] = vt
                    groups.append(v_group)
                return groups

            loop_cm = (tc.For_i(0, reps, 1) if reps > 1
                       else contextlib.nullcontext())
            with loop_cm:
                stage_chunk(0)
                for g in proj_groups(0):
                    g()
                for ci in range(NC):
                    blocks, q_lo, qwid, win_lo, win_hi = chunk_geom(ci)
                    qZ = qZ_bufs[ci % 2]
                    kT = kT_map[ci]
                    vts = vts_map[ci]
                    nextg = None
                    if ci + 1 < NC:
                        stage_chunk(ci + 1)
                        nextg = proj_groups(ci + 1)
                    nemit = 0
                    spliced = 0
                    nsplice = len(blocks)

                    def splice():
                        # emit next-chunk proj groups at fine grain so PE has
                        # independent matmuls at every chain re-entry point
                        nonlocal nemit, spliced
                        spliced += 1
                        if nextg is None:
                            return
                        hi_g = (len(nextg) * spliced) // nsplice
                        for g_fn in nextg[nemit:hi_g]:
                            g_fn()
                        nemit = hi_g

                    # atT supertile: per-block transposed attn written at 96j
                    # column offsets; o-proj consumes 128-col slices (M=128).
                    atT = smp.tile([128, 4, qwid], BF16, tag="atT", name="atT")
                    nst = qwid // 128
                    opro_after = {}
                    acc = 0
                    t = 0
                    for jj, j in enumerate(blocks):
                        acc += _qw(j)
                        while t < nst and acc >= 128 * (t + 1):
                            opro_after.setdefault(jj, []).append(t)
                            t += 1
                    for jj, j in enumerate(blocks):
                        qw = _qw(j)
                        qloc = QB * j - q_lo
                        kloc = (QB * j - HALF) - win_lo
                        pT = ptp.tile([128, H, QB], BF16, tag="pT", name="pT")
                        gps = []
                        for g in range(2):
                            st = st_ps.tile([128, 4, QB], F32, tag="st", name="st")
                            for hi in range(4):
                                h = 4 * g + hi
                                nc.tensor.matmul(
                                    st[:, hi, :qw],
                                    kT[:, h // 2, kloc:kloc + 128],
                                    qZ[:, h, qloc:qloc + qw],
                                    start=True, stop=True)
                            nc.scalar.activation(pT[:, 4 * g:4 * g + 4, :qw],
                                                 st[:, :, :qw], EXP,
                                                 bias=pen[:, j:j + 1], scale=1.0)
                            nc.gpsimd.tensor_tensor(
                                out=pT[:, 4 * g:4 * g + 4, :qw],
                                in0=pT[:, 4 * g:4 * g + 4, :qw],
                                in1=band[:, 0:qw].unsqueeze(1).to_broadcast(
                                    (128, 4, qw)),
                                op=MULT)
                        att = smp.tile([QB, D], BF16, tag="att", name="att")
                        attv = att.rearrange("q (h d) -> q h d", d=64)
                        rc = smp.tile([QB, H], F32, tag="rc", name="rc")
                        for g in range(2):
                            m2 = mm2_ps.tile([QB, 4, 66], F32, tag="m2", name="m2")
                            for hi in range(4):
                                h = 4 * g + hi
                                nc.tensor.matmul(m2[:qw, hi, :], pT[:, h, :qw],
                                                 vts[j][:, h, :],
                                                 start=True, stop=True)
                            gps.append(m2)
                        for g in range(2):
                            nc.vector.reciprocal(rc[:qw, 4 * g:4 * g + 4],
                                                 gps[g][:qw, :, 64])
                        for g in range(2):
                            nc.vector.tensor_tensor(
                                out=attv[:qw, 4 * g:4 * g + 4, :],
                                in0=gps[g][:qw, :, 0:64],
                                in1=rc[:qw, 4 * g:4 * g + 4].unsqueeze(2).to_broadcast(
                                    (qw, 4, 64)),
                                op=MULT)
                        # transpose attn -> [dcat, q], into the chunk supertile
                        tr = tr_ps.tile([128, 4, QB], BF16, tag="tr", name="tr")
                        for i in range(4):
                            nc.tensor.transpose(tr[:, i, :qw],
                                                att[:qw, 128 * i:128 * i + 128],
                                                iden[:qw, :qw])
                        nc.vector.tensor_copy(atT[:, :, qloc:qloc + qw],
                                              tr[:, :, :qw])
                        # output projection supertiles ready after this block
                        for st_i in opro_after.get(jj, ()):
                            sl = slice(128 * st_i, 128 * st_i + 128)
                            op = op_ps.tile([128, D], F32, tag="op", name="op")
                            for i in range(4):
                                nc.tensor.matmul(op[:, :], atT[:, i, sl],
                                                 wo_sb[:, i, :],
                                                 start=(i == 0), stop=(i == 3))
                            osb = outp.tile([128, D], F16, tag="osb", name="osb")
                            nc.scalar.copy(osb[:, :], op[:, :])
                            nc.sync.dma_start(
                                out_d[q_lo + 128 * st_i:q_lo + 128 * st_i + 128, :],
                                osb[:, :])
                        splice()

    nc.finalize()
    return nc


def _host_consts():
    rr = np.arange(128)[:, None]
    qq = np.arange(QB)[None, :]
    band = (((rr - qq) >= 0) & ((rr - qq) <= 32)).astype(np.float32)
    ident = np.eye(128, dtype=np.float32)
    return band, ident


def _get_nc(reps=None):
    reps = REPS if reps is None else reps
    key = reps
    if key not in _NCS:
        _NCS[key] = _build(reps)
    return _NCS[key]


def _prep_inmaps(query, key, value, mask, Wq, bq, Wk, bk, Wv, bv, Wo, bo):
    query = np.asarray(query, np.float32)
    key = np.asarray(key, np.float32)
    value = np.asarray(value, np.float32)
    mask = np.asarray(mask)
    Wq, bq = np.asarray(Wq, np.float32), np.asarray(bq, np.float32)
    Wk, bk = np.asarray(Wk, np.float32), np.asarray(bk, np.float32)
    Wv, bv = np.asarray(Wv, np.float32), np.asarray(bv, np.float32)
    Wo, bo = np.asarray(Wo, np.float32), np.asarray(bo, np.float32)

    band, ident = _host_consts()
    jj = np.arange(NB)[None, :]
    rr = np.arange(128)[:, None]
    pos = QB * jj - HALF + rr                      # [128, NB]
    valid = (pos >= 0) & (pos < S)
    posc = np.clip(pos, 0, S - 1)

    import ml_dtypes
    BF = ml_dtypes.bfloat16
    bqc = np.ascontiguousarray(bq.reshape(4, 128).T)
    bkc = np.ascontiguousarray(bk.reshape(4, 128).T)
    common = {
        "wqkvT": np.concatenate(
            [Wq.T, Wk.T, Wv.T], axis=0).astype(np.float16),
        "woT": np.ascontiguousarray(Wo.T).astype(BF),
        "cbf": np.concatenate([band, ident], axis=1).astype(BF),
        "zpad": np.zeros((128, 4, 64), np.float16),
    }
    from concurrent.futures import ThreadPoolExecutor

    def _one(b):
        pen = np.where(valid & ~mask[b][posc], 0.0, NEG).astype(np.float32)
        return dict(
            common,
            xqT=np.ascontiguousarray(query[b].T).astype(np.float16),
            xkT=np.ascontiguousarray(key[b].T).astype(np.float16),
            xvT=np.ascontiguousarray(value[b].T).astype(np.float16),
            cf32=np.concatenate([bqc, bkc, pen], axis=1).astype(np.float32),
        )

    with ThreadPoolExecutor(max_workers=8) as ex:
        in_maps = list(ex.map(_one, range(B)))
    return in_maps


def kernel(**inputs):
    from concourse.bass_utils import run_bass_kernel_spmd
    in_maps = _prep_inmaps(**inputs)
    res = run_bass_kernel_spmd(_get_nc(), in_maps, core_ids=list(range(8)))
    Wo = np.asarray(inputs["Wo"], np.float32)
    bv = np.asarray(inputs["bv"], np.float32)
    bo = np.asarray(inputs["bo"], np.float32)
    boeff = (Wo @ bv + bo).astype(np.float32)[None, :]
    out = np.stack([res.results[c]["out"].astype(np.float32) + boeff
                    for c in range(B)], axis=0)
    return out
